# revision 1
# baseline (speedup 1.0000x reference)
"""Trainium2 Bass kernel for the CML1D problem — v4 (f32r matmuls with
precision management, multi-engine balance, block-pair interleaving).

Math: 15 steps of  g' = bdm + 0.5 - (q0*s[i-1] + q1*s[i] + q2*s[i+1]),
with s = (g-0.5)^2 and bdm = beta*drive + csum.

The PE's fast fp32 mode (float32r, 1 cycle/col vs 4 for fp32) keeps only
an 11-bit mantissa, and this chaotic map amplifies per-step rounding by
~300x (max over 33M elements). Mitigations, validated in numpy against
the fp32 reference (max err ~1.1e-2 vs the 2e-2 gate):
  * center-prescale: state is stored as s^ = s/gamma with gamma chosen so
    the dominant center tap rtn(gamma*q1)/gamma == q1 exactly; the f32r
    rounding of W then only perturbs the small side taps.
  * bdm is stored shifted by its range center (c0b), shrinking its f32r
    rounding 4x; c0b is re-added via free scalar/bias slots.
  * steps 0 and 1 run the conv with exact fp32 matmuls (4 cyc/col), so
    the most-amplified early injections see full fp32; the state is f32r
    from s1 onward.
Per column-chunk pipes for steps >= 2 (Pool cannot access PSUM and
cannot scale a square, so it only helps with setup):
  A: psum = W~^T s^ + I^T bdm~; s^' = ACT Square(sqrt(1/g)*psum + bias)
  D: psum = W~^T s^; v = psum + c0b + bdm~ (DVE stt); s^' = ACT Square(scale*v)
  G: like D but the square also on DVE: stt (v*invg)*v.
The pipe table balances PE/ACT/DVE busy time (LP-tuned). Two blocks
(a "group" = 128 batch rows) are chunk-interleaved; the two groups run
back to back. Final step emits g = v + 0.5 in bf16 (rel err 2e-3 << gate);
the reference clip to [1e-4, 1-1e-4] is a provable no-op beyond ~1e-4
absolute (g stays in (0, 0.98)), so it is skipped.
"""
import sys

sys.path.insert(0, "/opt/trn_rl_repo")
from contextlib import ExitStack

import numpy as np

import concourse.tile as tile
from concourse import bacc, mybir
from concourse.bass_utils import run_bass_kernel_spmd

F32 = mybir.dt.float32
F32R = mybir.dt.float32r
BF16 = mybir.dt.bfloat16
AF = mybir.ActivationFunctionType
OP = mybir.AluOpType

R, EPS, BETA, STEPS = 3.9, 0.3, 0.15, 15

P = 128          # partitions / window size
H = 5            # halo per side
S = P - 2 * H    # window stride = 118
N_CORES = 8

LAT = 16384
BATCH = 2048
BPC = BATCH // N_CORES   # 256 rows per core
BB = 64                  # batch rows per block
NBLK = BPC // BB

CHUNK = 1024             # psum drain chunk (2 banks)
NEXACT = 2               # steps using exact fp32 matmuls
RTN_FORCE = 0.0          # hardware f32r writes round-to-nearest; forcing hurts

# pipe per (block, chunk); blocks are paired; per-pair mix ~ A 61%, G 27%,
# D 12% balances PE/ACT/DVE
PIPES = [
    ["A", "G", "A", "D", "A", "G", "A", "A", "G"],
    ["A", "G", "A", "D", "A", "A", "G", "A", "A"],
    ["A", "G", "A", "D", "A", "G", "A", "A", "G"],
    ["A", "G", "A", "D", "A", "A", "G", "A", "A"],
]

SKIP_EXCHANGE = False
SKIP_SETUP = False
SKEW = 0                 # intra-pair block skew (b1 lags b0)


def _rtn12(x):
    """Round fp32 to the f32r grid (11-bit mantissa), round-to-nearest."""
    x = np.ascontiguousarray(np.asarray(x, np.float32)).reshape(-1)
    xi = x.view(np.uint32)
    r = (((xi.astype(np.uint64) + 0x800) & 0xFFFFF000).astype(np.uint32)).view(
        np.float32
    )
    return np.float32(r[0]) if r.size == 1 else r


def _nw(lat):
    nw = -(-lat // S)
    assert lat - S * (nw - 1) >= H, "ragged seam too small for halo exchange"
    return nw


def _chunks(cols):
    out = []
    c0 = 0
    while c0 < cols:
        out.append((c0, min(c0 + CHUNK, cols)))
        c0 = min(c0 + CHUNK, cols)
    return out


def _mm_slices(w):
    """Moving-dim slices: matmul outputs must not cross 512-col PSUM bank
    boundaries, so slice at multiples of 512. A sub-256 tail runs at the
    slow f32r rate but is rare (one 192-col slice per 8896-col block)."""
    out = [512] * (w // 512)
    if w % 512:
        out.append(w % 512)
    return out


def build_nc(lat=LAT, bb=BB, nblk=NBLK, steps=STEPS):
    nw = _nw(lat)
    cols = nw * bb
    seam = lat - S * (nw - 1)
    chunks = _chunks(cols)
    nch = len(chunks)
    pipes = [[PIPES[b % len(PIPES)][ci % len(PIPES[0])] for ci in range(nch)]
             for b in range(nblk)]

    nc = bacc.Bacc("TRN2", target_bir_lowering=False, debug=False)
    drive_w = nc.dram_tensor("drive_w", [nblk, P, cols], F32, kind="ExternalInput")
    wmat_r = nc.dram_tensor("wmat_r", [P, P], F32R, kind="ExternalInput")
    wmat_x = nc.dram_tensor("wmat_x", [P, P], F32, kind="ExternalInput")
    eye = nc.dram_tensor("eye", [P, P], F32R, kind="ExternalInput")
    # per-partition scalar columns: beta', csum'-c0b, sqi*c0b, -sqi/2
    consts = nc.dram_tensor("consts", [P, 8], F32, kind="ExternalInput")
    out_w = nc.dram_tensor("out_w", [nblk, S, cols], BF16, kind="ExternalOutput")

    sc = build_nc._host_scalars
    SQI = sc["sqrt_invg"]
    INVG = sc["invg"]
    C0B = sc["c0b"]

    with tile.TileContext(nc) as tc, ExitStack() as ctx:
        const_pool = ctx.enter_context(tc.tile_pool(name="constp", bufs=1))
        sp = ctx.enter_context(tc.tile_pool(name="state", bufs=2))
        ldp = ctx.enter_context(tc.tile_pool(name="ld", bufs=7))
        vp = ctx.enter_context(tc.tile_pool(name="v", bufs=6))
        gp = ctx.enter_context(tc.tile_pool(name="g", bufs=7))
        pp = ctx.enter_context(tc.tile_pool(name="ps", bufs=4, space="PSUM"))

        w_r = const_pool.tile([P, P], F32R, tag="wr")
        nc.sync.dma_start(w_r[:], wmat_r.ap())
        w_x = const_pool.tile([P, P], F32, tag="wx")
        nc.sync.dma_start(w_x[:], wmat_x.ap())
        i_r = const_pool.tile([P, P], F32R, tag="ir")
        nc.sync.dma_start(i_r[:], eye.ap())
        c_t = const_pool.tile([P, 8], F32, tag="c")
        nc.sync.dma_start(c_t[:], consts.ap())
        beta_ap = c_t[:, 0:1]
        csum_ap = c_t[:, 1:2]
        biasA_ap = c_t[:, 2:3]
        bias0_ap = c_t[:, 3:4]

        state = {}  # blk -> (s, bdm)
        lds = {}    # blk -> list of drive chunk scratch tiles

        def emit_loads(blk, lo=0, hi=None):
            if blk not in state:
                s = sp.tile([P, cols], F32R, tag="s", name=f"s_b{blk}")
                bdm = sp.tile([P, cols], F32R, tag="bdm", name=f"bdm_b{blk}")
                state[blk] = (s, bdm)
                lds[blk] = [None] * nch
            hi = nch if hi is None else hi
            for ci in range(lo, hi):
                c0, c1 = chunks[ci]
                lt = ldp.tile([P, CHUNK], F32, tag="ld", name=f"ld{ci}_b{blk}")
                nc.sync.dma_start(lt[:, :c1 - c0], drive_w.ap()[blk, :, c0:c1])
                lds[blk][ci] = lt

        def emit_setup_chunk(blk, ci):
            """s0 (F32, exact, scaled by invg) into a scratch; bdm~ (F32R)
            from the drive chunk. Returns the s0 scratch tile."""
            c0, c1 = chunks[ci]
            s, bdm = state[blk]
            lt = lds[blk][ci]
            w = c1 - c0
            s0 = vp.tile([P, CHUNK], F32, tag="v", name=f"s0_{ci}_b{blk}")
            # s0 = (sqi*drive - sqi/2)^2 = invg*(drive-0.5)^2
            nc.scalar.activation(
                s0[:, :w], lt[:, :w], AF.Square, bias=bias0_ap, scale=SQI
            )
            # bdm~ = beta*drive + (csum - c0b)
            nc.vector.tensor_scalar(
                bdm[:, c0:c1], lt[:, :w], beta_ap, csum_ap,
                op0=OP.mult, op1=OP.add,
            )
            return s0

        def emit_step_chunk(blk, t, ci, s0=None):
            c0, c1 = chunks[ci]
            w = c1 - c0
            s, bdm = state[blk]
            exact = t < NEXACT
            pipe = pipes[blk][ci]
            last = t == steps - 1
            if exact:
                pipe = "D"
            if last:
                pipe = {"A": "Ag", "D": "Dg", "G": "Dg", "F": "Dg"}[pipe]
            with_i = pipe in ("A", "Ag")
            pt = pp.tile([P, CHUNK], F32, tag="ps", name=f"pt{t}_{ci}_b{blk}")
            m0 = 0
            for msz in _mm_slices(w):
                psl = pt[:, m0:m0 + msz]
                if exact:
                    if t == 0:
                        ssl = s0[:, m0:m0 + msz]
                    else:
                        ssl = s[:, c0 + m0:c0 + m0 + msz].bitcast(F32)
                    nc.tensor.matmul(psl, w_x[:], ssl, start=True, stop=True)
                else:
                    ssl = s[:, c0 + m0:c0 + m0 + msz]
                    if with_i:
                        bsl = bdm[:, c0 + m0:c0 + m0 + msz]
                        nc.tensor.matmul(psl, w_r[:], ssl, start=True, stop=False)
                        nc.tensor.matmul(psl, i_r[:], bsl, start=False, stop=True)
                    else:
                        nc.tensor.matmul(psl, w_r[:], ssl, start=True, stop=True)
                m0 += msz

            if pipe == "A":
                nc.scalar.activation(
                    s[:, c0:c1], pt[:, :w], AF.Square, bias=biasA_ap, scale=SQI
                )
            elif pipe in ("D", "G", "F"):
                v = vp.tile([P, CHUNK], F32, tag="v", name=f"v{t}_{ci}_b{blk}")
                nc.vector.scalar_tensor_tensor(
                    v[:, :w], pt[:, :w], C0B, bdm[:, c0:c1],
                    op0=OP.add, op1=OP.add,
                )
                if pipe == "D":
                    nc.scalar.activation(
                        s[:, c0:c1], v[:, :w], AF.Square, bias=0.0, scale=SQI
                    )
                elif pipe == "G":
                    nc.vector.scalar_tensor_tensor(
                        s[:, c0:c1], v[:, :w], INVG, v[:, :w],
                        op0=OP.mult, op1=OP.mult,
                    )
                else:
                    # Pool square (sbuf only, no scale) + DVE 2x rescale
                    w2 = vp.tile([P, CHUNK], F32, tag="v", name=f"w{t}_{ci}_b{blk}")
                    nc.gpsimd.tensor_tensor(w2[:, :w], v[:, :w], v[:, :w], op=OP.mult)
                    nc.vector.tensor_scalar(
                        s[:, c0:c1], w2[:, :w], INVG, 0.0, op0=OP.mult, op1=OP.add
                    )
            else:
                # final step: g = psum (+bdm~) + c0b + 0.5, bf16, no clip
                g = gp.tile([P, CHUNK], BF16, tag="g", name=f"g{ci}_b{blk}")
                if pipe == "Ag":
                    nc.scalar.activation(
                        g[:, :w], pt[:, :w], AF.Copy, bias=C0B + 0.5, scale=1.0
                    )
                else:
                    nc.vector.scalar_tensor_tensor(
                        g[:, :w], pt[:, :w], C0B + 0.5, bdm[:, c0:c1],
                        op0=OP.add, op1=OP.add,
                    )
                nc.sync.dma_start(out_w.ap()[blk, :, c0:c1], g[H:H + S, :w])

        def emit_exchange(blk):
            s, _ = state[blk]
            lastw = nw - 1
            p0 = S * (nw - 1) - lat + P - H
            nc.sync.dma_start(
                s[0:H, 0:bb], s[seam:seam + H, lastw * bb:(lastw + 1) * bb]
            )
            nc.sync.dma_start(
                s[P - H:P, lastw * bb:(lastw + 1) * bb], s[p0:p0 + H, 0:bb]
            )
            mid = (nw // 2) * bb
            nc.sync.dma_start(s[0:H, bb:mid + bb], s[P - 2 * H:P - H, 0:mid])
            nc.sync.dma_start(
                s[0:H, mid + bb:nw * bb],
                s[P - 2 * H:P - H, mid:(nw - 1) * bb],
            )
            nc.sync.dma_start(s[P - H:P, 0:mid], s[H:2 * H, bb:mid + bb])
            nc.sync.dma_start(
                s[P - H:P, mid:(nw - 1) * bb], s[H:2 * H, mid + bb:nw * bb]
            )

        def emit_block_step(blk, t):
            for ci in range(nch):
                s0 = None
                if t == 0 and not SKIP_SETUP:
                    s0 = emit_setup_chunk(blk, ci)
                emit_step_chunk(blk, t, ci, s0=s0)
            if (t + 1) % H == 0 and t != steps - 1 and not SKIP_EXCHANGE:
                emit_exchange(blk)

        groups = [tuple(range(g, min(g + 2, nblk))) for g in range(0, nblk, 2)]
        for gi, grp in enumerate(groups):
            if gi == 0:
                for blk in grp:
                    emit_loads(blk)
            # block 2 of the pair lags by SKEW steps so its PE-heavy exact
            # steps overlap block 1's ACT/DVE-heavy steady steps
            for vt in range(steps + SKEW * (len(grp) - 1)):
                if gi + 1 < len(groups):
                    nxt = groups[gi + 1]
                    if vt == 2 * H:
                        emit_loads(nxt[0], 0, 4)
                    elif vt == 2 * H + 2 and len(nxt) > 1:
                        emit_loads(nxt[1], 0, 4)
                for bi, blk in enumerate(grp):
                    t = vt - SKEW * bi
                    if 0 <= t < steps:
                        emit_block_step(blk, t)
                if (vt == steps + SKEW * (len(grp) - 1) - 1
                        and gi + 1 < len(groups)):
                    for nb in groups[gi + 1]:
                        emit_loads(nb, 4, nch)

    nc.compile()
    return nc


build_nc._host_scalars = {"sqrt_invg": 1.0, "invg": 1.0, "c0b": 0.0}


def _host_constants(K):
    K = np.asarray(K, dtype=np.float64)
    q0 = np.float32((1.0 - BETA) * EPS * K[0] * R)
    q1 = np.float32((1.0 - BETA) * (1.0 - EPS + EPS * K[1]) * R)
    q2 = np.float32((1.0 - BETA) * EPS * K[2] * R)
    csum = np.float32(0.25 * (float(q0) + float(q1) + float(q2)) - 0.5)

    frc = 1.0 + RTN_FORCE
    gamma = float(np.float64(_rtn12(q1)) / np.float64(q1))
    invg = np.float32(1.0 / gamma)
    sqi = np.float32(np.sqrt(1.0 / gamma))
    c0b = np.float32(float(csum) + 0.075)

    qh = [np.float32(_rtn12(np.float32(gamma * float(q)))) for q in (q0, q1, q2)]

    def banded(taps):
        W = np.zeros((P, P), np.float32)
        for p in range(1, P - 1):
            W[p - 1, p] = -taps[0]
            W[p, p] = -taps[1]
            W[p + 1, p] = -taps[2]
        return W

    Wr = banded(qh)
    # exact-step taps act on the gamma-scaled state s^ = s/gamma
    Wx = banded([np.float32(gamma * float(q)) for q in (q0, q1, q2)])

    consts = np.zeros((P, 8), np.float32)
    consts[:, 0] = np.float32(BETA) * frc
    consts[:, 1] = (float(csum) - float(c0b)) * frc
    consts[:, 2] = float(sqi) * float(c0b)
    consts[:, 3] = -float(sqi) * 0.5
    scalars = {
        "sqrt_invg": float(sqi) * float(np.sqrt(frc)),
        "invg": float(invg) * frc,
        "c0b": float(c0b),
    }
    return Wr, Wx, consts, scalars


def _window(d, lat, bb, nblk):
    nw = _nw(lat)
    c_idx = np.arange(nw) * S
    p_idx = np.arange(P)
    idx = (c_idx[:, None] + p_idx[None, :] - H) % lat  # [nw, P]
    win = d[:, idx]  # [rows, nw, P]
    win = win.reshape(nblk, bb, nw, P).transpose(0, 3, 2, 1)
    return np.ascontiguousarray(win).reshape(nblk, P, nw * bb)


def _unwindow(o, lat, bb, nblk):
    nw = _nw(lat)
    o = o.reshape(nblk, S, nw, bb).transpose(0, 3, 2, 1)
    o = o.reshape(nblk * bb, nw * S)
    return o[:, :lat]


_NC_CACHE = {}
TRACE = False
LAST_RESULT = None


def _get_nc(lat, bb, nblk, steps, scalars):
    key = (lat, bb, nblk, steps, tuple(sorted(scalars.items())))
    if key not in _NC_CACHE:
        build_nc._host_scalars = scalars
        _NC_CACHE[key] = build_nc(lat, bb, nblk, steps)
    return _NC_CACHE[key]


def kernel(drive, K):
    drive = np.asarray(drive, dtype=np.float32)
    K = np.asarray(K, dtype=np.float32)
    b, mid, lat = drive.shape
    d2 = drive.reshape(b, lat)
    Wr, Wx, consts, scalars = _host_constants(K)
    nc = _get_nc(LAT, BB, NBLK, STEPS, scalars)
    eye = np.eye(P, dtype=np.float32)

    in_maps = []
    for c in range(N_CORES):
        dcore = d2[c * BPC:(c + 1) * BPC]
        in_maps.append(
            {
                "drive_w": _window(dcore, LAT, BB, NBLK),
                "wmat_r": Wr,
                "wmat_x": Wx,
                "eye": eye,
                "consts": consts,
            }
        )
    global LAST_RESULT
    res = None
    for attempt in range(3):
        try:
            res = run_bass_kernel_spmd(
                nc, in_maps, core_ids=list(range(N_CORES)), trace=TRACE
            )
            break
        except Exception:
            if attempt == 2:
                raise
            import time

            try:
                import jax

                jax.clear_caches()
                from jax._src import xla_bridge

                xla_bridge._clear_backends()
            except Exception:
                pass
            time.sleep(5.0)
    LAST_RESULT = res
    outs = [
        _unwindow(np.asarray(res.results[c]["out_w"]).astype(np.float32), LAT, BB, NBLK)
        for c in range(N_CORES)
    ]
    out = np.concatenate(outs, axis=0).reshape(b, mid, lat).astype(np.float32)
    return out

